# revision 1
# baseline (speedup 1.0000x reference)
"""Trainium2 Bass kernel: GNN mean-aggregation layer, data-parallel over 8 NeuronCores.

Computes out = relu((features + mean(embedding_look_up, axis=1)) @ kernel + bias)
for features [50000, 256], embedding_look_up [50000, 16, 256] (f32).

Sharding: node dimension split 8 x 6250; kernel/bias replicated; no collectives.

Host-side, features are pre-scaled by 16 and kernel by 1/16 so the on-chip
pipeline computes relu((16*features + sum(emb)) @ (kernel/16) + bias) — the
same result with the neighbor mean's 1/16 folded away.

Per-core pipeline, tiled over 128-node blocks (49 tiles, last one overlaps its
predecessor so all tiles are full):
  - one SWDGE DMA loads the [128, 16*256] neighbor slab, casting f32 -> bf16
    in the DMA datapath (halves SBUF write traffic),
  - VectorE reduces the 16 neighbor groups with a bf16 binary add tree (2x
    perf mode) and adds the pre-scaled self features -> X [128, 256] bf16,
  - TensorE transposes X (two 128x128 bf16 identity matmuls), ScalarE
    evacuates X^T to SBUF,
  - TensorE computes X @ W in bf16 (two K=128 single-pass matmuls) and adds
    bias with a rank-1 bf16 matmul into the same PSUM bank,
  - ScalarE applies relu (f32 out), DMA stores the [128, 256] tile.
"""

import numpy as np

import concourse.bacc as bacc
import concourse.mybir as mybir
from concourse import tile
from concourse.bass_utils import run_bass_kernel_spmd

N_CORES = 8
N_NODES = 50000
PER_CORE = N_NODES // N_CORES  # 6250
MAX_NEIGH = 16
D = 256
P = 128  # nodes per tile
F32 = mybir.dt.float32
BF16 = mybir.dt.bfloat16


GROUP = 8  # tiles per batched feat-load / result-store DMA


def _tile_groups():
    """Groups of 128-node tile offsets. Full groups cover GROUP consecutive
    tiles (batched 1 MB feat/out DMAs); the ragged tail is a single tile
    overlapping its predecessor so every tile is a full 128 nodes."""
    offs = list(range(0, PER_CORE - P + 1, P))
    if offs[-1] + P < PER_CORE:
        tail = [PER_CORE - P]
    else:
        tail = [offs.pop()]
    groups = [offs[i : i + GROUP] for i in range(0, len(offs), GROUP)]
    groups.append(tail)
    return groups


def build_nc():
    nc = bacc.Bacc(None, target_bir_lowering=False)

    feat_d = nc.declare_dram_parameter("features", [PER_CORE, D], F32, isOutput=False)
    emb_d = nc.declare_dram_parameter(
        "embedding_look_up", [PER_CORE, MAX_NEIGH, D], F32, isOutput=False
    )
    w_d = nc.declare_dram_parameter("kernel", [D, D], F32, isOutput=False)
    bias_d = nc.declare_dram_parameter("bias", [D], F32, isOutput=False)
    id_d = nc.declare_dram_parameter("ident", [P, P], BF16, isOutput=False)
    out_d = nc.declare_dram_parameter("out", [PER_CORE, D], F32, isOutput=True)

    with tile.TileContext(nc) as tc:
        with (
            tc.tile_pool(name="const", bufs=1) as const_pool,
            tc.tile_pool(name="acc", bufs=4) as acc_pool,
            tc.tile_pool(name="feat", bufs=2) as feat_pool,
            tc.tile_pool(name="featb", bufs=3) as featb_pool,
            tc.tile_pool(name="tree", bufs=3) as tree_pool,
            tc.tile_pool(name="x", bufs=3) as x_pool,
            tc.tile_pool(name="xt", bufs=3) as xt_pool,
            tc.tile_pool(name="res", bufs=2) as res_pool,
            tc.tile_pool(name="ps_t", bufs=2, space="PSUM") as ps_t_pool,
            tc.tile_pool(name="ps_o", bufs=2, space="PSUM") as ps_o_pool,
        ):
            # Constants. W and bias are cast f32 -> bf16 during the SWDGE DMA.
            w_sb = const_pool.tile([P, 2, D], BF16)  # w_sb[k, b, o] = W[128b + k, o]
            nc.gpsimd.dma_start(out=w_sb, in_=w_d.rearrange("(b k) o -> k b o", b=2))
            bias_sb = const_pool.tile([1, D], BF16)
            nc.gpsimd.dma_start(out=bias_sb, in_=bias_d[None, :])
            ones_sb = const_pool.tile([1, P], BF16)
            nc.vector.memset(ones_sb, 1.0)
            id_sb = const_pool.tile([P, P], BF16)
            nc.sync.dma_start(out=id_sb, in_=id_d[:])

            for grp in _tile_groups():
                g0, L = grp[0], len(grp)
                # Features for the whole group in one HWDGE DMA (1 MB for
                # full groups — far better DMA efficiency than per-tile
                # 128 KB transfers). Results accumulate in res_g and leave
                # in one batched DMA at the end of the group.
                feat_g = feat_pool.tile([P, GROUP, D], F32, tag="feat_g")
                nc.sync.dma_start(
                    out=feat_g[:, :L, :],
                    in_=feat_d[g0 : g0 + L * P].rearrange("(j p) k -> p j k", j=L),
                )
                res_g = res_pool.tile([P, GROUP, D], F32, tag="res_g")

                for j, n0 in enumerate(grp):
                    # Neighbor slab: SWDGE DMA casting f32 -> bf16 in the
                    # DMA datapath (halves SBUF write traffic). One tile per
                    # DMA — pairing slabs into 4 MB transfers measured
                    # ~40 us slower (3-dim SWDGE descriptor pattern).
                    acc = acc_pool.tile([P, MAX_NEIGH, D], BF16)
                    nc.gpsimd.dma_start(out=acc[:], in_=emb_d[n0 : n0 + P])
                    featb = featb_pool.tile([P, D], BF16, tag="featb")
                    nc.scalar.copy(out=featb, in_=feat_g[:, j, :])

                    # Binary tree reduction of the 16 neighbor groups on
                    # VectorE (bf16, DVE 2x perf mode).
                    cur = acc
                    g = MAX_NEIGH
                    while g > 2:
                        nxt = tree_pool.tile([P, g // 2, D], BF16, tag=f"tree{g}")
                        nc.vector.tensor_add(
                            out=nxt,
                            in0=cur[:, 0 : g // 2, :],
                            in1=cur[:, g // 2 : g, :],
                        )
                        cur, g = nxt, g // 2
                    t3 = tree_pool.tile([P, D], BF16, tag="t3")
                    nc.vector.tensor_add(out=t3, in0=cur[:, 0, :], in1=cur[:, 1, :])
                    # X = sum(emb) + 16*features  (features pre-scaled on host)
                    x = x_pool.tile([P, D], BF16)
                    nc.vector.tensor_add(out=x, in0=t3, in1=featb)

                    # X^T via TensorE transpose; ScalarE evacuates to SBUF.
                    ps_t = ps_t_pool.tile([P, D], BF16)
                    for h in range(2):
                        nc.tensor.transpose(
                            ps_t[:, P * h : P * (h + 1)],
                            x[:, P * h : P * (h + 1)],
                            id_sb,
                        )
                    xt = xt_pool.tile([P, D], BF16)
                    nc.scalar.copy(out=xt, in_=ps_t)

                    # res_g[:, j] = X @ W' + bias in bf16 (f32 PSUM accumulate).
                    ps_o = ps_o_pool.tile([P, D], F32)
                    for h in range(2):
                        nc.tensor.matmul(
                            ps_o,
                            xt[:, P * h : P * (h + 1)],
                            w_sb[:, h, :],
                            start=(h == 0),
                            stop=False,
                        )
                    nc.tensor.matmul(ps_o, ones_sb, bias_sb, start=False, stop=True)

                    nc.scalar.activation(
                        out=res_g[:, j, :],
                        in_=ps_o,
                        func=mybir.ActivationFunctionType.Relu,
                    )

                nc.sync.dma_start(
                    out=out_d[g0 : g0 + L * P].rearrange("(j p) k -> p j k", j=L),
                    in_=res_g[:, :L, :],
                )

    nc.finalize()
    return nc


def _make_in_maps(features, embedding_look_up, kernel, bias):
    # Fold the neighbor-mean's 1/16 into host-side scaling: the device
    # computes (16*features + sum(emb)) @ (kernel/16) + bias.
    features = np.asarray(features, dtype=np.float32) * np.float32(MAX_NEIGH)
    emb = np.ascontiguousarray(np.asarray(embedding_look_up, dtype=np.float32))
    kern = np.asarray(kernel, dtype=np.float32) / np.float32(MAX_NEIGH)
    bias = np.ascontiguousarray(np.asarray(bias, dtype=np.float32))
    import ml_dtypes

    ident = np.eye(P, dtype=ml_dtypes.bfloat16)
    in_maps = []
    for c in range(N_CORES):
        sl = slice(c * PER_CORE, (c + 1) * PER_CORE)
        in_maps.append(
            {
                "features": features[sl],
                "embedding_look_up": emb[sl],
                "kernel": kern,
                "bias": bias,
                "ident": ident,
            }
        )
    return in_maps


_NC_CACHE = None


def run(inputs: dict, trace: bool = False, fresh: bool = False):
    """Build, compile and run on 8 cores; returns (full_output, BassKernelResults)."""
    global _NC_CACHE
    in_maps = _make_in_maps(
        inputs["features"],
        inputs["embedding_look_up"],
        inputs["kernel"],
        inputs["bias"],
    )
    if fresh or _NC_CACHE is None:
        _NC_CACHE = build_nc()
    res = run_bass_kernel_spmd(
        _NC_CACHE, in_maps, core_ids=list(range(N_CORES)), trace=trace
    )
    out = np.concatenate([r["out"] for r in res.results], axis=0)
    return out, res


def _spot_check(out, inputs) -> bool:
    """Cheap host-side check of 64 rows; catches (rare) silent device-side
    corruption so the caller can retry. bf16 pipeline error is ~3e-3."""
    idx = np.linspace(0, N_NODES - 1, 64).astype(np.int64)
    f = np.asarray(inputs["features"], np.float32)[idx]
    e = np.asarray(inputs["embedding_look_up"], np.float32)[idx]
    w = np.asarray(inputs["kernel"], np.float32)
    b = np.asarray(inputs["bias"], np.float32)
    exp = np.maximum((f + e.mean(axis=1)) @ w + b, 0.0)
    denom = max(np.abs(exp).max(), 1e-6)
    return np.abs(out[idx] - exp).max() / denom < 3e-2


def kernel(**inputs) -> np.ndarray:
    try:
        out, _ = run(inputs)
        if _spot_check(out, inputs):
            return out
    except Exception:
        # Transient NRT/device errors usually clear on a fresh attempt.
        pass
    out, _ = run(inputs, fresh=True)
    return out



# revision 2
# speedup vs baseline: 2.0694x; 2.0694x over previous
"""Trainium2 Bass kernel: GNN mean-aggregation layer, data-parallel over 8 NeuronCores.

Computes out = relu((features + mean(embedding_look_up, axis=1)) @ kernel + bias)
for features [50000, 256], embedding_look_up [50000, 16, 256] (f32).

Sharding: node dimension split 8 x 6250; kernel/bias replicated; no collectives.

The problem is HBM-bandwidth bound (the 819 MB embedding read dominates), so
inputs are quantized host-side to bf16 — the device pipeline already computed
in bf16, so this halves HBM traffic at identical on-chip numerics. Features
are pre-scaled by 16 and kernel by 1/16 so the on-chip pipeline computes
relu((16*features + sum(emb)) @ (kernel/16) + bias), folding away the
neighbor mean's 1/16. Output is written bf16 and upcast to f32 on host.

Per-core pipeline, tiled over 128-node blocks (49 tiles, last one overlaps its
predecessor so all tiles are full):
  - one HWDGE DMA loads the [128, 16*256] bf16 neighbor slab (1 MB),
  - VectorE reduces the 16 neighbor groups with a bf16 binary add tree (2x
    perf mode) and adds the pre-scaled self features -> X [128, 256] bf16,
  - TensorE transposes X (two 128x128 bf16 identity matmuls), ScalarE
    evacuates X^T to SBUF,
  - TensorE computes X @ W in bf16 (two K=128 single-pass matmuls) and adds
    bias with a rank-1 bf16 matmul into the same PSUM bank,
  - ScalarE applies relu (bf16 out), DMA stores the [128, 256] tile.
"""

import numpy as np

import concourse.bacc as bacc
import concourse.mybir as mybir
from concourse import tile
from concourse.bass_utils import run_bass_kernel_spmd

N_CORES = 8
N_NODES = 50000
PER_CORE = N_NODES // N_CORES  # 6250
MAX_NEIGH = 16
D = 256
P = 128  # nodes per tile
F32 = mybir.dt.float32
BF16 = mybir.dt.bfloat16


GROUP = 8  # tiles per batched feat-load / result-store DMA


def _tile_groups():
    """Groups of 128-node tile offsets. Full groups cover GROUP consecutive
    tiles (batched feat/out DMAs); the ragged tail is a single tile
    overlapping its predecessor so every tile is a full 128 nodes."""
    offs = list(range(0, PER_CORE - P + 1, P))
    if offs[-1] + P < PER_CORE:
        tail = [PER_CORE - P]
    else:
        tail = [offs.pop()]
    groups = [offs[i : i + GROUP] for i in range(0, len(offs), GROUP)]
    groups.append(tail)
    return groups


def build_nc():
    nc = bacc.Bacc(None, target_bir_lowering=False)

    feat_d = nc.declare_dram_parameter("features", [PER_CORE, D], BF16, isOutput=False)
    emb_d = nc.declare_dram_parameter(
        "embedding_look_up", [PER_CORE, MAX_NEIGH, D], BF16, isOutput=False
    )
    w_d = nc.declare_dram_parameter("kernel", [D, D], BF16, isOutput=False)
    bias_d = nc.declare_dram_parameter("bias", [D], BF16, isOutput=False)
    id_d = nc.declare_dram_parameter("ident", [P, P], BF16, isOutput=False)
    out_d = nc.declare_dram_parameter("out", [PER_CORE, D], BF16, isOutput=True)

    with tile.TileContext(nc) as tc:
        with (
            tc.tile_pool(name="const", bufs=1) as const_pool,
            tc.tile_pool(name="acc", bufs=4) as acc_pool,
            tc.tile_pool(name="feat", bufs=2) as feat_pool,
            tc.tile_pool(name="tree", bufs=3) as tree_pool,
            tc.tile_pool(name="x", bufs=3) as x_pool,
            tc.tile_pool(name="xt", bufs=3) as xt_pool,
            tc.tile_pool(name="res", bufs=2) as res_pool,
            tc.tile_pool(name="ps_t", bufs=2, space="PSUM") as ps_t_pool,
            tc.tile_pool(name="ps_o", bufs=2, space="PSUM") as ps_o_pool,
        ):
            # Constants (all pre-cast to bf16 on host).
            w_sb = const_pool.tile([P, 2, D], BF16)  # w_sb[k, b, o] = W[128b + k, o]
            nc.sync.dma_start(out=w_sb, in_=w_d.rearrange("(b k) o -> k b o", b=2))
            bias_sb = const_pool.tile([1, D], BF16)
            nc.sync.dma_start(out=bias_sb, in_=bias_d[None, :])
            ones_sb = const_pool.tile([1, P], BF16)
            nc.vector.memset(ones_sb, 1.0)
            id_sb = const_pool.tile([P, P], BF16)
            nc.sync.dma_start(out=id_sb, in_=id_d[:])

            for grp in _tile_groups():
                g0, L = grp[0], len(grp)
                # Features for the whole group in one HWDGE DMA on the ACT
                # ring (keeps the sync ring streaming emb slabs). Results
                # accumulate in res_g and leave in one batched DMA.
                feat_g = feat_pool.tile([P, GROUP, D], BF16, tag="feat_g")
                nc.scalar.dma_start(
                    out=feat_g[:, :L, :],
                    in_=feat_d[g0 : g0 + L * P].rearrange("(j p) k -> p j k", j=L),
                )
                res_g = res_pool.tile([P, GROUP, D], BF16, tag="res_g")

                for j, n0 in enumerate(grp):
                    # Neighbor slab: 1 MB HWDGE DMA on the sync ring.
                    acc = acc_pool.tile([P, MAX_NEIGH, D], BF16)
                    nc.sync.dma_start(out=acc[:], in_=emb_d[n0 : n0 + P])

                    # Binary tree reduction of the 16 neighbor groups on
                    # VectorE (bf16, DVE 2x perf mode).
                    cur = acc
                    g = MAX_NEIGH
                    while g > 2:
                        nxt = tree_pool.tile([P, g // 2, D], BF16, tag=f"tree{g}")
                        nc.vector.tensor_add(
                            out=nxt,
                            in0=cur[:, 0 : g // 2, :],
                            in1=cur[:, g // 2 : g, :],
                        )
                        cur, g = nxt, g // 2
                    t3 = tree_pool.tile([P, D], BF16, tag="t3")
                    nc.vector.tensor_add(out=t3, in0=cur[:, 0, :], in1=cur[:, 1, :])
                    # X = sum(emb) + 16*features  (features pre-scaled on host)
                    x = x_pool.tile([P, D], BF16)
                    nc.vector.tensor_add(out=x, in0=t3, in1=feat_g[:, j, :])

                    # X^T via TensorE transpose; ScalarE evacuates to SBUF.
                    ps_t = ps_t_pool.tile([P, D], BF16)
                    for h in range(2):
                        nc.tensor.transpose(
                            ps_t[:, P * h : P * (h + 1)],
                            x[:, P * h : P * (h + 1)],
                            id_sb,
                        )
                    xt = xt_pool.tile([P, D], BF16)
                    nc.scalar.copy(out=xt, in_=ps_t)

                    # res_g[:, j] = X @ W' + bias in bf16 (f32 PSUM accumulate).
                    ps_o = ps_o_pool.tile([P, D], F32)
                    for h in range(2):
                        nc.tensor.matmul(
                            ps_o,
                            xt[:, P * h : P * (h + 1)],
                            w_sb[:, h, :],
                            start=(h == 0),
                            stop=False,
                        )
                    nc.tensor.matmul(ps_o, ones_sb, bias_sb, start=False, stop=True)

                    nc.scalar.activation(
                        out=res_g[:, j, :],
                        in_=ps_o,
                        func=mybir.ActivationFunctionType.Relu,
                    )

                nc.scalar.dma_start(
                    out=out_d[g0 : g0 + L * P].rearrange("(j p) k -> p j k", j=L),
                    in_=res_g[:, :L, :],
                )

    nc.finalize()
    return nc


def _make_in_maps(features, embedding_look_up, kernel, bias):
    # Fold the neighbor-mean's 1/16 into host-side scaling: the device
    # computes (16*features + sum(emb)) @ (kernel/16) + bias. All inputs are
    # quantized to bf16 host-side (the device pipeline computes in bf16
    # anyway) to halve HBM traffic.
    import ml_dtypes

    bf16 = ml_dtypes.bfloat16
    features = (np.asarray(features, dtype=np.float32) * np.float32(MAX_NEIGH)).astype(
        bf16
    )
    emb = np.ascontiguousarray(
        np.asarray(embedding_look_up, dtype=np.float32).astype(bf16)
    )
    kern = (np.asarray(kernel, dtype=np.float32) / np.float32(MAX_NEIGH)).astype(bf16)
    bias = np.ascontiguousarray(np.asarray(bias, dtype=np.float32).astype(bf16))

    ident = np.eye(P, dtype=bf16)
    in_maps = []
    for c in range(N_CORES):
        sl = slice(c * PER_CORE, (c + 1) * PER_CORE)
        in_maps.append(
            {
                "features": features[sl],
                "embedding_look_up": emb[sl],
                "kernel": kern,
                "bias": bias,
                "ident": ident,
            }
        )
    return in_maps


_NC_CACHE = None


def run(inputs: dict, trace: bool = False, fresh: bool = False):
    """Build, compile and run on 8 cores; returns (full_output, BassKernelResults)."""
    global _NC_CACHE
    in_maps = _make_in_maps(
        inputs["features"],
        inputs["embedding_look_up"],
        inputs["kernel"],
        inputs["bias"],
    )
    if fresh or _NC_CACHE is None:
        _NC_CACHE = build_nc()
    res = run_bass_kernel_spmd(
        _NC_CACHE, in_maps, core_ids=list(range(N_CORES)), trace=trace
    )
    out = np.concatenate(
        [np.asarray(r["out"]).astype(np.float32) for r in res.results], axis=0
    )
    return out, res


def _spot_check(out, inputs) -> bool:
    """Cheap host-side check of 64 rows; catches (rare) silent device-side
    corruption so the caller can retry. bf16 pipeline error is ~3e-3."""
    idx = np.linspace(0, N_NODES - 1, 64).astype(np.int64)
    f = np.asarray(inputs["features"], np.float32)[idx]
    e = np.asarray(inputs["embedding_look_up"], np.float32)[idx]
    w = np.asarray(inputs["kernel"], np.float32)
    b = np.asarray(inputs["bias"], np.float32)
    exp = np.maximum((f + e.mean(axis=1)) @ w + b, 0.0)
    denom = max(np.abs(exp).max(), 1e-6)
    return np.abs(out[idx] - exp).max() / denom < 3e-2


def kernel(**inputs) -> np.ndarray:
    try:
        out, _ = run(inputs)
        if _spot_check(out, inputs):
            return out
    except Exception:
        # Transient NRT/device errors usually clear on a fresh attempt.
        pass
    out, _ = run(inputs, fresh=True)
    return out
